# revision 23
# baseline (speedup 1.0000x reference)
"""KernelConv2D (per-pixel dynamic 5x5 depthwise conv) on 8 TRN2 NeuronCores.

Problem: out[b,c,h,w] = sum_{i,j} x_edgepad[b,c,h+i,w+j] * K[b,c,i,j,h,w]
with input [4,32,128,128] f32 and kernel [4,800,128,128] f32 (800 = 32*25).

Sharding: every (b,c) plane is independent, so flatten to 128 planes and put
the plane index on the SBUF partition axis. Each core takes 16 output ROWS of
all 128 planes (row-sharding). With (h, w) both living in the free dimension,
both conv shifts are constant free-dim offsets -> the 5x5 taps of the input
window are expressed as a single overlapping access pattern, no halo exchange
or partition-shifted copies on device. Host pre-pads the input with edge
replication and slices per-core row bands (incl. 2-row halo).

The problem is memory-bound on the 200MB kernel tensor. The harness gate is
rel_err < 2e-2, so X/K/out travel as bf16 (host converts): products of two
bf16-rounded normals summed over 25 taps give rel_l2 ~ 3e-3, an order under
the gate. Per-core HBM traffic drops to K 13.1MB + X 0.67MB + out 0.52MB
~= 14.3MB -> ~38us of stream at the measured ~400 GB/s/core burst rate
(f32 baseline: ~98us; this kernel: ~55us, the rest being fixed NEFF
preamble/teardown (~15us) plus ramp and tail).

Steady-state balance (2-row chunk, ~4.1us DMA pace): DVE does 2 full-25-tap
row products (2x-packed, 3.65us) + a 2-pair pre-add (0.42us); PE accumulates
the remaining 23 bf16 segments via identity matmuls into PSUM (~3us incl the
per-matmul eye LDWEIGHTS). Hard-won scheduling rules, all measured on HW:
the pre-add must follow the products (kt recycling gates the K stream on the
chunk's last product); the PE group must put the product-gated taps FIRST
and the pre-add-gated dt segments LAST (dt-first serializes PE behind the
whole DVE chunk and builds a ~7us tail backlog); GpSimd must stay idle (its
SBUF traffic slows DVE products ~40%); deeper tile-pool prefetch (kpool>3)
slows the stream via SBUF-port pressure; finer DMA splits than taps-0-9/
10-24 slow the whole stream (more, smaller packets).
"""

import sys

import ml_dtypes
import numpy as np

sys.path.insert(0, "/opt/trn_rl_repo")

import concourse.bacc as bacc
import concourse.bass as bass
import concourse.tile as tile
from concourse import mybir
from concourse.ap import AP
from concourse.bass_utils import run_bass_kernel_spmd

N_CORES = 8
B, C, H, W, KS = 4, 32, 128, 128, 5
NPLANES = B * C          # 128 -> partition axis
NTAPS = KS * KS          # 25
ROWS_PER_CORE = H // N_CORES   # 16
ROWS_PER_CHUNK = 2
# Trailing 1-row chunks halve the compute tail after the last K byte lands.
CHUNK_ROWS = [2, 2, 2, 2, 2, 2, 2, 1, 1]
CHUNK_STARTS = [0, 2, 4, 6, 8, 10, 12, 14, 15]
NCHUNK = len(CHUNK_ROWS)
FDW = ROWS_PER_CHUNK * W                   # max output elems per chunk-partition
XW = W + KS - 1                            # 132 padded row width
XROWS = ROWS_PER_CORE + KS - 1             # 20 rows incl halo
F32 = mybir.dt.float32
BF16 = mybir.dt.bfloat16
NPBF16 = ml_dtypes.bfloat16

# Reduction: the otherwise-idle TensorEngine sums the tap-product segments
# with identity matmuls accumulating into one PSUM bank (bf16 1.0*x promotes
# exactly; PSUM accumulation is f32 add). ScalarE evacuates PSUM -> SBUF.

_compiled = None


def _build_program():
    nc = bacc.Bacc(
        "TRN2",
        target_bir_lowering=False,
        debug=False,
        enable_asserts=False,
        num_devices=N_CORES,
        # The program never branches on core id (per-core data differs on the
        # host side only); dropping the tensor removes the per-engine
        # partition-id register loads from the preamble.
        enable_partition_id=False,
    )
    # Host pre-arranges k as [plane][chunk][tap][h2][w] so each chunk load is
    # one contiguous per-partition run (few DMA descriptors, near line rate).
    xd = nc.declare_dram_parameter("x", [NPLANES, XROWS * XW], BF16, isOutput=False)
    kd = nc.declare_dram_parameter(
        "k", [NPLANES, NTAPS * ROWS_PER_CORE * W], BF16, isOutput=False
    )
    od = nc.declare_dram_parameter(
        "o", [NPLANES, ROWS_PER_CORE * W], BF16, isOutput=True
    )
    ed = nc.declare_dram_parameter("eye", [NPLANES, NPLANES], BF16, isOutput=False)

    with tile.TileContext(nc) as tc:
        with (
            tc.tile_pool(name="xpool", bufs=1) as xpool,
            tc.tile_pool(name="epool", bufs=1) as epool,
            tc.tile_pool(name="kpool", bufs=3) as kpool,
            tc.tile_pool(name="ppool", bufs=2) as ppool,
            tc.tile_pool(name="dpool", bufs=2) as dpool,
            tc.tile_pool(name="spool", bufs=3, space="PSUM") as spool,
            tc.tile_pool(name="opool", bufs=3) as opool,
        ):
            # Whole padded input band for this core, resident for the kernel.
            # Only rows 0-7 (chunks 0-1) block startup; the rest loads during
            # chunk 1. The K stream is issued FIRST (inside the chunk loop
            # below): its 13.1MB paces the whole kernel, so its first byte
            # should be the first byte on the wire; X head and eye slot in
            # behind it and still land before anything consumes them.
            xt = xpool.tile([NPLANES, XROWS * XW], BF16)
            et = epool.tile([NPLANES, NPLANES], BF16)
            xt_ap = xt[:]
            xt_pdim = xt_ap.ap[0]  # (partition step, 128)

            for ch in range(NCHUNK):
                h0 = CHUNK_STARTS[ch]
                rows = CHUNK_ROWS[ch]
                fdw = rows * W
                kt = kpool.tile([NPLANES, NTAPS * FDW], BF16, tag="kt")
                base = NTAPS * W * h0
                sseg = KS * fdw
                if ch == 1:
                    # X body rides the ACT/store ring (idle until ~13us):
                    # the sync ring then carries pure K, so the last K byte
                    # (which gates the tail) lands ~1.9us earlier.
                    nc.scalar.dma_start(
                        out=xt[:, 8 * XW :], in_=xd.ap()[:, 8 * XW :]
                    )
                # Two sub-loads per chunk (taps 0-9 / 10-24). Finer splits
                # were measured slower (more, smaller packets slow the whole
                # stream); coarser ones lengthen the tail.
                nc.sync.dma_start(
                    out=kt[:, 0 : 2 * sseg],
                    in_=kd.ap()[:, base : base + 2 * sseg],
                )
                nc.sync.dma_start(
                    out=kt[:, 2 * sseg : KS * sseg],
                    in_=kd.ap()[:, base + 2 * sseg : base + KS * sseg],
                )
                if ch == 0:
                    # X head + eye ride the ACT/store ring too; they land by
                    # ~10.5us, before the first products need them (those
                    # also gate on chunk 0's K at ~11us).
                    nc.scalar.dma_start(
                        out=xt[:, 0 : 8 * XW], in_=xd.ap()[:, 0 : 8 * XW]
                    )
                    nc.scalar.dma_start(out=et[:], in_=ed.ap())
                pt = ppool.tile([NPLANES, NTAPS * FDW], BF16, tag="pt")

                # Products on the DVE. kt/pt are tap-major per chunk (tap t
                # at offset t*fdw, row r at +r*W within it), X rows are
                # XW-strided, so a (i, j, w) 3-dim overlapping window covers
                # many taps of one output row in ONE op. Middle chunks use
                # one op per row over all 25 taps (fewest ops -> DVE ~13%
                # under the DMA pace, so the K stream is never DVE-gated);
                # the last chunk splits taps 0-9 / 10-24 per row so only the
                # 10-24 op (+15 matmuls) gates on the final K bytes.
                kt_ap = kt[:]
                pt_ap = pt[:]

                def prod_row(r, i0, i1):
                    n = i1 - i0
                    k_view = AP(
                        kt_ap.tensor,
                        kt_ap.offset + (i0 * KS) * fdw + r * W,
                        [kt_ap.ap[0], (KS * fdw, n), (fdw, KS), (1, W)],
                    )
                    p_view = AP(
                        pt_ap.tensor,
                        pt_ap.offset + (i0 * KS) * fdw + r * W,
                        [pt_ap.ap[0], (KS * fdw, n), (fdw, KS), (1, W)],
                    )
                    x_view = AP(
                        xt_ap.tensor,
                        xt_ap.offset + (h0 + r + i0) * XW,
                        [xt_pdim, (XW, n), (1, KS), (1, W)],
                    )
                    nc.vector.tensor_mul(p_view, k_view, x_view)

                # DVE order: products, then the 2-pair pre-add (taps 0-1 +
                # 2-3). The pre-add comes AFTER the products (mid-sequence it
                # measurably stalls the K stream via delayed kt recycling)
                # and stays on the DVE (GpSimd: 2.7us/chunk and its SBUF
                # traffic slows DVE products ~40% — measured). Only 2 pairs:
                # enough to bring the PE's 23 remaining segments under the
                # DMA pace without pushing the DVE over it.
                last = ch == NCHUNK - 1
                for r in range(rows):
                    if last:
                        prod_row(r, 0, 2)
                    else:
                        prod_row(r, 0, KS)
                dt = dpool.tile([NPLANES, 2 * FDW], BF16, tag="dt")
                nc.vector.tensor_add(
                    dt[:, 0 : 2 * fdw], pt[:, 0 : 2 * fdw], pt[:, 2 * fdw : 4 * fdw]
                )
                if last:
                    # Tail: per-i products so each 5-matmul PE group fires as
                    # soon as its slice of the final sub-load lands.
                    for i in range(2, KS):
                        for r in range(rows):
                            prod_row(r, i, i + 1)

                # TensorE: identity matmuls accumulate the segments into one
                # PSUM bank (bf16 is one PE pass; accumulation f32). dt pairs
                # + taps 4-9 first (ready with the early sub-load on the last
                # chunk), taps 10-24 last.
                st = spool.tile([NPLANES, FDW], F32, tag="st")
                if last:
                    # dt + taps 4-9 are ready with the early sub-load; only
                    # taps 10-24 gate on the final K bytes.
                    segs = [dt[:, t * fdw : (t + 1) * fdw] for t in range(2)]
                    segs += [pt[:, t * fdw : (t + 1) * fdw] for t in range(4, NTAPS)]
                else:
                    # Taps first: they only need the products, so the PE
                    # engages as soon as the chunk's products land instead of
                    # waiting for the pre-add (the chunk's LAST DVE op).
                    segs = [pt[:, t * fdw : (t + 1) * fdw] for t in range(4, NTAPS)]
                    segs += [dt[:, t * fdw : (t + 1) * fdw] for t in range(2)]
                for t, s in enumerate(segs):
                    nc.tensor.matmul(
                        st[:, 0:fdw],
                        et[:],
                        s,
                        start=(t == 0),
                        stop=(t == len(segs) - 1),
                    )

                # ScalarE: evacuate PSUM -> SBUF (f32 -> bf16), then store.
                ot = opool.tile([NPLANES, FDW], BF16, tag="ot")
                nc.scalar.copy(ot[:, 0:fdw], st[:, 0:fdw])
                # Stores go on the ACT HWDGE ring so a compute-gated store
                # never blocks K loads queued on the sync ring (FIFO/ring).
                nc.scalar.dma_start(
                    out=od.ap()[:, h0 * W : h0 * W + fdw], in_=ot[:, 0:fdw]
                )

    nc.compile()
    return nc


def _get_program():
    global _compiled
    if _compiled is None:
        _compiled = _build_program()
    return _compiled


def _shard_inputs(input: np.ndarray, kernel: np.ndarray):
    x = np.ascontiguousarray(input, dtype=np.float32).reshape(NPLANES, H, W)
    xp = np.pad(x, ((0, 0), (2, 2), (2, 2)), mode="edge").astype(NPBF16)
    k = np.ascontiguousarray(kernel, dtype=np.float32).reshape(
        NPLANES, NTAPS, H, W
    ).astype(NPBF16)
    eye = np.eye(NPLANES, dtype=NPBF16)
    in_maps = []
    for c in range(N_CORES):
        r0 = c * ROWS_PER_CORE
        # [plane][tap][16 rows][w] -> per-chunk [plane][tap][rows][w] blocks,
        # concatenated so each chunk is one contiguous per-plane run.
        ks = k[:, :, r0 : r0 + ROWS_PER_CORE, :]
        blocks = [
            ks[:, :, s : s + n, :].reshape(NPLANES, NTAPS * n * W)
            for s, n in zip(CHUNK_STARTS, CHUNK_ROWS)
        ]
        kc = np.ascontiguousarray(np.concatenate(blocks, axis=1))
        in_maps.append(
            {
                "x": np.ascontiguousarray(
                    xp[:, r0 : r0 + XROWS, :]
                ).reshape(NPLANES, XROWS * XW),
                "k": kc,
                "eye": eye,
            }
        )
    return in_maps


last_results = None  # BassKernelResults of the most recent run (for profiling)


def kernel(input: np.ndarray, kernel: np.ndarray, _trace: bool = False):
    global last_results
    nc = _get_program()
    in_maps = _shard_inputs(input, kernel)
    res = run_bass_kernel_spmd(nc, in_maps, list(range(N_CORES)), trace=_trace)
    last_results = res
    out = np.empty((NPLANES, H, W), dtype=np.float32)
    for c in range(N_CORES):
        out[:, c * ROWS_PER_CORE : (c + 1) * ROWS_PER_CORE, :] = res.results[c][
            "o"
        ].astype(np.float32).reshape(NPLANES, ROWS_PER_CORE, W)
    return out.reshape(B, C, H, W)


if __name__ == "__main__":
    rng = np.random.default_rng(0)
    inp = rng.standard_normal((B, C, H, W), dtype=np.float32)
    kern = rng.standard_normal((B, C * NTAPS, H, W), dtype=np.float32)
    out = kernel(inp, kern)
    print("ran ok", out.shape, out.dtype)


# revision 24
# speedup vs baseline: 1.0299x; 1.0299x over previous
"""KernelConv2D (per-pixel dynamic 5x5 depthwise conv) on 8 TRN2 NeuronCores.

Problem: out[b,c,h,w] = sum_{i,j} x_edgepad[b,c,h+i,w+j] * K[b,c,i,j,h,w]
with input [4,32,128,128] f32 and kernel [4,800,128,128] f32 (800 = 32*25).

Sharding: every (b,c) plane is independent, so flatten to 128 planes and put
the plane index on the SBUF partition axis. Each core takes 16 output ROWS of
all 128 planes (row-sharding). With (h, w) both living in the free dimension,
both conv shifts are constant free-dim offsets -> the 5x5 taps of the input
window are expressed as a single overlapping access pattern, no halo exchange
or partition-shifted copies on device. Host pre-pads the input with edge
replication and slices per-core row bands (incl. 2-row halo).

The problem is memory-bound on the 200MB kernel tensor. The harness gate is
rel_err < 2e-2, so X/K/out travel as bf16 (host converts): products of two
bf16-rounded normals summed over 25 taps give rel_l2 ~ 3e-3, an order under
the gate. Per-core HBM traffic drops to K 13.1MB + X 0.67MB + out 0.52MB
~= 14.3MB -> ~38us of stream at the measured ~400 GB/s/core burst rate
(f32 baseline: ~98us; this kernel: ~55us, the rest being fixed NEFF
preamble/teardown (~15us) plus ramp and tail).

Steady-state balance (2-row chunk, ~4.1us DMA pace): DVE does 2 full-25-tap
row products (2x-packed, 3.65us) + a 2-pair pre-add (0.42us); PE accumulates
the remaining 23 bf16 segments via identity matmuls into PSUM (~3us incl the
per-matmul eye LDWEIGHTS). Hard-won scheduling rules, all measured on HW:
the pre-add must follow the products (kt recycling gates the K stream on the
chunk's last product); the PE group must put the product-gated taps FIRST
and the pre-add-gated dt segments LAST (dt-first serializes PE behind the
whole DVE chunk and builds a ~7us tail backlog); GpSimd must stay idle (its
SBUF traffic slows DVE products ~40%); deeper tile-pool prefetch (kpool>3)
slows the stream via SBUF-port pressure; finer DMA splits than taps-0-9/
10-24 slow the whole stream (more, smaller packets).
"""

import sys

import ml_dtypes
import numpy as np

sys.path.insert(0, "/opt/trn_rl_repo")

import concourse.bacc as bacc
import concourse.bass as bass
import concourse.tile as tile
from concourse import mybir
from concourse.ap import AP
from concourse.bass_utils import run_bass_kernel_spmd

N_CORES = 8
B, C, H, W, KS = 4, 32, 128, 128, 5
NPLANES = B * C          # 128 -> partition axis
NTAPS = KS * KS          # 25
ROWS_PER_CORE = H // N_CORES   # 16
ROWS_PER_CHUNK = 2
# Trailing 1-row chunks halve the compute tail after the last K byte lands.
CHUNK_ROWS = [2, 2, 2, 2, 2, 2, 2, 1, 1]
CHUNK_STARTS = [0, 2, 4, 6, 8, 10, 12, 14, 15]
NCHUNK = len(CHUNK_ROWS)
FDW = ROWS_PER_CHUNK * W                   # max output elems per chunk-partition
XW = W + KS - 1                            # 132 padded row width
XROWS = ROWS_PER_CORE + KS - 1             # 20 rows incl halo
F32 = mybir.dt.float32
BF16 = mybir.dt.bfloat16
NPBF16 = ml_dtypes.bfloat16

# Reduction: the otherwise-idle TensorEngine sums the tap-product segments
# with identity matmuls accumulating into one PSUM bank (bf16 1.0*x promotes
# exactly; PSUM accumulation is f32 add). ScalarE evacuates PSUM -> SBUF.

_compiled = None


def _build_program():
    nc = bacc.Bacc(
        "TRN2",
        target_bir_lowering=False,
        debug=False,
        enable_asserts=False,
        num_devices=N_CORES,
        # The program never branches on core id (per-core data differs on the
        # host side only); dropping the tensor removes the per-engine
        # partition-id register loads from the preamble.
        enable_partition_id=False,
    )
    # Host pre-arranges k as [plane][chunk][tap][h2][w] so each chunk load is
    # one contiguous per-partition run (few DMA descriptors, near line rate).
    xd = nc.declare_dram_parameter("x", [NPLANES, XROWS * XW], BF16, isOutput=False)
    kd = nc.declare_dram_parameter(
        "k", [NPLANES, NTAPS * ROWS_PER_CORE * W], BF16, isOutput=False
    )
    od = nc.declare_dram_parameter(
        "o", [NPLANES, ROWS_PER_CORE * W], BF16, isOutput=True
    )
    ed = nc.declare_dram_parameter("eye", [NPLANES, NPLANES], BF16, isOutput=False)

    with tile.TileContext(nc) as tc:
        with (
            tc.tile_pool(name="xpool", bufs=1) as xpool,
            tc.tile_pool(name="epool", bufs=1) as epool,
            tc.tile_pool(name="kpool", bufs=3) as kpool,
            tc.tile_pool(name="ppool", bufs=2) as ppool,
            tc.tile_pool(name="dpool", bufs=2) as dpool,
            tc.tile_pool(name="spool", bufs=3, space="PSUM") as spool,
            tc.tile_pool(name="opool", bufs=3) as opool,
        ):
            # Whole padded input band for this core, resident for the kernel.
            # Only rows 0-7 (chunks 0-1) block startup; the rest loads during
            # chunk 1. The K stream is issued FIRST (inside the chunk loop
            # below): its 13.1MB paces the whole kernel, so its first byte
            # should be the first byte on the wire; X head and eye slot in
            # behind it and still land before anything consumes them.
            xt = xpool.tile([NPLANES, XROWS * XW], BF16)
            et = epool.tile([NPLANES, NPLANES], BF16)
            xt_ap = xt[:]
            xt_pdim = xt_ap.ap[0]  # (partition step, 128)

            for ch in range(NCHUNK):
                h0 = CHUNK_STARTS[ch]
                rows = CHUNK_ROWS[ch]
                fdw = rows * W
                kt = kpool.tile([NPLANES, NTAPS * FDW], BF16, tag="kt")
                base = NTAPS * W * h0
                sseg = KS * fdw
                if ch == 1:
                    # X body rides the ACT/store ring (idle until ~13us):
                    # the sync ring then carries pure K, so the last K byte
                    # (which gates the tail) lands ~1.9us earlier.
                    nc.scalar.dma_start(
                        out=xt[:, 8 * XW :], in_=xd.ap()[:, 8 * XW :]
                    )
                # Two sub-loads per chunk (taps 0-9 / 10-24). Finer splits
                # were measured slower (more, smaller packets slow the whole
                # stream); coarser ones lengthen the tail.
                nc.sync.dma_start(
                    out=kt[:, 0 : 2 * sseg],
                    in_=kd.ap()[:, base : base + 2 * sseg],
                )
                nc.sync.dma_start(
                    out=kt[:, 2 * sseg : KS * sseg],
                    in_=kd.ap()[:, base + 2 * sseg : base + KS * sseg],
                )
                if ch == 0:
                    # X head + eye ride the ACT/store ring too; they land by
                    # ~10.5us, before the first products need them (those
                    # also gate on chunk 0's K at ~11us).
                    nc.scalar.dma_start(
                        out=xt[:, 0 : 8 * XW], in_=xd.ap()[:, 0 : 8 * XW]
                    )
                    nc.scalar.dma_start(out=et[:], in_=ed.ap())
                pt = ppool.tile([NPLANES, NTAPS * FDW], BF16, tag="pt")

                # Products on the DVE. kt/pt are tap-major per chunk (tap t
                # at offset t*fdw, row r at +r*W within it), X rows are
                # XW-strided, so a (i, j, w) 3-dim overlapping window covers
                # many taps of one output row in ONE op. Middle chunks use
                # one op per row over all 25 taps (fewest ops -> DVE ~13%
                # under the DMA pace, so the K stream is never DVE-gated);
                # the last chunk splits taps 0-9 / 10-24 per row so only the
                # 10-24 op (+15 matmuls) gates on the final K bytes.
                kt_ap = kt[:]
                pt_ap = pt[:]

                def prod_row(r, i0, i1):
                    n = i1 - i0
                    k_view = AP(
                        kt_ap.tensor,
                        kt_ap.offset + (i0 * KS) * fdw + r * W,
                        [kt_ap.ap[0], (KS * fdw, n), (fdw, KS), (1, W)],
                    )
                    p_view = AP(
                        pt_ap.tensor,
                        pt_ap.offset + (i0 * KS) * fdw + r * W,
                        [pt_ap.ap[0], (KS * fdw, n), (fdw, KS), (1, W)],
                    )
                    x_view = AP(
                        xt_ap.tensor,
                        xt_ap.offset + (h0 + r + i0) * XW,
                        [xt_pdim, (XW, n), (1, KS), (1, W)],
                    )
                    nc.vector.tensor_mul(p_view, k_view, x_view)

                # DVE order: products, then the 2-pair pre-add (taps 0-1 +
                # 2-3). The pre-add comes AFTER the products (mid-sequence it
                # measurably stalls the K stream via delayed kt recycling)
                # and stays on the DVE (GpSimd: 2.7us/chunk and its SBUF
                # traffic slows DVE products ~40% — measured). Only 2 pairs:
                # enough to bring the PE's 23 remaining segments under the
                # DMA pace without pushing the DVE over it.
                last = ch >= NCHUNK - 2
                for r in range(rows):
                    if last:
                        prod_row(r, 0, 2)
                    else:
                        prod_row(r, 0, KS)
                dt = dpool.tile([NPLANES, 2 * FDW], BF16, tag="dt")
                nc.vector.tensor_add(
                    dt[:, 0 : 2 * fdw], pt[:, 0 : 2 * fdw], pt[:, 2 * fdw : 4 * fdw]
                )
                if last:
                    # Tail: per-i products so each 5-matmul PE group fires as
                    # soon as its slice of the final sub-load lands.
                    for i in range(2, KS):
                        for r in range(rows):
                            prod_row(r, i, i + 1)

                # TensorE: identity matmuls accumulate the segments into one
                # PSUM bank (bf16 is one PE pass; accumulation f32). dt pairs
                # + taps 4-9 first (ready with the early sub-load on the last
                # chunk), taps 10-24 last.
                st = spool.tile([NPLANES, FDW], F32, tag="st")
                if last:
                    # dt + taps 4-9 are ready with the early sub-load; only
                    # taps 10-24 gate on the final K bytes.
                    segs = [dt[:, t * fdw : (t + 1) * fdw] for t in range(2)]
                    segs += [pt[:, t * fdw : (t + 1) * fdw] for t in range(4, NTAPS)]
                else:
                    # Taps first: they only need the products, so the PE
                    # engages as soon as the chunk's products land instead of
                    # waiting for the pre-add (the chunk's LAST DVE op).
                    segs = [pt[:, t * fdw : (t + 1) * fdw] for t in range(4, NTAPS)]
                    segs += [dt[:, t * fdw : (t + 1) * fdw] for t in range(2)]
                for t, s in enumerate(segs):
                    nc.tensor.matmul(
                        st[:, 0:fdw],
                        et[:],
                        s,
                        start=(t == 0),
                        stop=(t == len(segs) - 1),
                    )

                # ScalarE: evacuate PSUM -> SBUF (f32 -> bf16), then store.
                ot = opool.tile([NPLANES, FDW], BF16, tag="ot")
                nc.scalar.copy(ot[:, 0:fdw], st[:, 0:fdw])
                # Stores go on the ACT HWDGE ring so a compute-gated store
                # never blocks K loads queued on the sync ring (FIFO/ring).
                nc.scalar.dma_start(
                    out=od.ap()[:, h0 * W : h0 * W + fdw], in_=ot[:, 0:fdw]
                )

    nc.compile()
    return nc


def _get_program():
    global _compiled
    if _compiled is None:
        _compiled = _build_program()
    return _compiled


def _shard_inputs(input: np.ndarray, kernel: np.ndarray):
    x = np.ascontiguousarray(input, dtype=np.float32).reshape(NPLANES, H, W)
    xp = np.pad(x, ((0, 0), (2, 2), (2, 2)), mode="edge").astype(NPBF16)
    k = np.ascontiguousarray(kernel, dtype=np.float32).reshape(
        NPLANES, NTAPS, H, W
    ).astype(NPBF16)
    eye = np.eye(NPLANES, dtype=NPBF16)
    in_maps = []
    for c in range(N_CORES):
        r0 = c * ROWS_PER_CORE
        # [plane][tap][16 rows][w] -> per-chunk [plane][tap][rows][w] blocks,
        # concatenated so each chunk is one contiguous per-plane run.
        ks = k[:, :, r0 : r0 + ROWS_PER_CORE, :]
        blocks = [
            ks[:, :, s : s + n, :].reshape(NPLANES, NTAPS * n * W)
            for s, n in zip(CHUNK_STARTS, CHUNK_ROWS)
        ]
        kc = np.ascontiguousarray(np.concatenate(blocks, axis=1))
        in_maps.append(
            {
                "x": np.ascontiguousarray(
                    xp[:, r0 : r0 + XROWS, :]
                ).reshape(NPLANES, XROWS * XW),
                "k": kc,
                "eye": eye,
            }
        )
    return in_maps


last_results = None  # BassKernelResults of the most recent run (for profiling)


def kernel(input: np.ndarray, kernel: np.ndarray, _trace: bool = False):
    global last_results
    nc = _get_program()
    in_maps = _shard_inputs(input, kernel)
    res = run_bass_kernel_spmd(nc, in_maps, list(range(N_CORES)), trace=_trace)
    last_results = res
    out = np.empty((NPLANES, H, W), dtype=np.float32)
    for c in range(N_CORES):
        out[:, c * ROWS_PER_CORE : (c + 1) * ROWS_PER_CORE, :] = res.results[c][
            "o"
        ].astype(np.float32).reshape(NPLANES, ROWS_PER_CORE, W)
    return out.reshape(B, C, H, W)


if __name__ == "__main__":
    rng = np.random.default_rng(0)
    inp = rng.standard_normal((B, C, H, W), dtype=np.float32)
    kern = rng.standard_normal((B, C * NTAPS, H, W), dtype=np.float32)
    out = kernel(inp, kern)
    print("ran ok", out.shape, out.dtype)
